# revision 14
# baseline (speedup 1.0000x reference)
"""LocalAggregator (GAT-style dual-relation message passing) on 8 TRN2 cores.

Math (per batch b, N=100 nodes, D=128):
  e_k[i,j]   = sum_d h[i,d]*h[j,d]*A[d,k]      (k=0..2)   -- symmetric in (i,j)
  b_k[i,j]   = sum_d h[i,d]*h[j,d]*Bm[d,k]     (k=0..8)   -- symmetric
  alpha      = softmax_j( leaky( e_{adj-1} ) masked adj==0 )
  alpha_beh  = softmax_j( leaky( b_{beh-1} ) masked beh==0 )
  out        = 0.5*alpha@h + 0.5*alpha_beh@h

Device strategy (data-parallel, 16 batches/core, groups of 4):
  - planes are symmetric -> select with TRANSPOSED adjacency to directly build
    nT [j, i] (lhsT of the aggregation matmul). No on-chip transposes of planes.
  - no softmax max-subtraction (scores are O(5)); invalid entries get -1e5 via
    acc init so exp()->0.
  - denominator via ones-column matmul; normalization = one fused
    scalar_tensor_tensor per batch.
"""

import numpy as np

import concourse.bass as bass
import concourse.bacc as bacc
import concourse.mybir as mybir
import concourse.tile as tile
from concourse.bass_utils import run_bass_kernel_spmd

F32 = mybir.dt.float32
U8 = mybir.dt.uint8

B, N, D = 128, 100, 128
NCORES = 8
BPC = B // NCORES          # 16 batches per core
GRP = 4                    # batches per group
NGRP = BPC // GRP          # 4 groups
HW = D + 4                 # 132: h row + ones col (+3 pad for alignment)
ALPHA = 0.2
MASKV = -1.0e5

_NC_CACHE = {}


def _build_nc():
    nc = bacc.Bacc()
    hplus = nc.declare_dram_parameter("hplus", [N, BPC * HW], F32, isOutput=False)
    htr = nc.declare_dram_parameter("htr", [D, BPC * N], F32, isOutput=False)
    adjt = nc.declare_dram_parameter("adjt", [N, BPC * N], U8, isOutput=False)
    beht = nc.declare_dram_parameter("beht", [N, BPC * N], U8, isOutput=False)
    acat = nc.declare_dram_parameter("acat", [D, 12], F32, isOutput=False)
    out = nc.declare_dram_parameter("out", [N, BPC * D], F32, isOutput=True)

    with tile.TileContext(nc) as tc:
        with (
            tc.tile_pool(name="const", bufs=1) as constp,
            tc.tile_pool(name="io", bufs=2) as iop,
            tc.tile_pool(name="gk", bufs=3) as gkp,
            tc.tile_pool(name="work", bufs=2) as workp,
            tc.tile_pool(name="eqp", bufs=3) as eqp,
            tc.tile_pool(name="plps", bufs=2, space="PSUM") as plps,
            tc.tile_pool(name="aggps", bufs=1, space="PSUM") as aggps,
        ):
            acat_sb = constp.tile([D, 12], F32)
            nc.sync.dma_start(out=acat_sb, in_=acat[:, :])
            # 2.0 so den = 2*sum and 1/den directly gives the 0.5 blend factor
            ones_sb = constp.tile([N, 1], F32)
            nc.vector.memset(ones_sb, 2.0)

            for g in range(NGRP):
                hp = iop.tile([N, GRP * HW], F32, tag="hp")
                nc.sync.dma_start(out=hp, in_=hplus[:, g * GRP * HW:(g + 1) * GRP * HW])
                adt = iop.tile([N, GRP * N], U8, tag="adt")
                nc.sync.dma_start(out=adt, in_=adjt[:, g * GRP * N:(g + 1) * GRP * N])
                bet = iop.tile([N, GRP * N], U8, tag="bet")
                nc.sync.dma_start(out=bet, in_=beht[:, g * GRP * N:(g + 1) * GRP * N])

                # hT for the 4 batches: [128(d), 4*100(j)], host-pretransposed
                ht4 = iop.tile([D, GRP * N], F32, tag="ht4")
                nc.sync.dma_start(out=ht4, in_=htr[:, g * GRP * N:(g + 1) * GRP * N])

                accA = workp.tile([N, GRP * N], F32, tag="accA")
                nc.vector.memset(accA, MASKV)
                accB = workp.tile([N, GRP * N], F32, tag="accB")
                nc.vector.memset(accB, MASKV)

                for k in range(12):
                    gk = gkp.tile([D, GRP * N], F32, tag="gk")
                    nc.scalar.activation(
                        gk, ht4, mybir.ActivationFunctionType.Copy,
                        scale=acat_sb[:, k:k + 1],
                    )
                    pl = plps.tile([N, GRP * N], F32, tag="pl")
                    for b in range(GRP):
                        nc.tensor.matmul(
                            pl[:, b * N:(b + 1) * N],
                            ht4[:, b * N:(b + 1) * N],
                            gk[:, b * N:(b + 1) * N],
                        )
                    eq = eqp.tile([N, GRP * N], U8, tag="eq")
                    if k < 3:
                        nc.gpsimd.tensor_scalar(
                            eq, adt, k + 1, None, mybir.AluOpType.is_equal
                        )
                        nc.vector.copy_predicated(accA, eq, pl)
                    else:
                        nc.gpsimd.tensor_scalar(
                            eq, bet, k - 2, None, mybir.AluOpType.is_equal
                        )
                        nc.vector.copy_predicated(accB, eq, pl)

                # n = exp(leaky_0.2(acc)) = max(exp(acc), exp(0.2*acc));
                # invalid entries stay exp(-1e5) = 0.  (ACT Lrelu hardcodes
                # slope 0.01, so the max-of-exps identity is used instead.)
                nAT = workp.tile([N, GRP * N], F32, tag="nAT")
                nA2 = workp.tile([N, GRP * N], F32, tag="nA2")
                nc.scalar.activation(nAT, accA, mybir.ActivationFunctionType.Exp)
                nc.scalar.activation(
                    nA2, accA, mybir.ActivationFunctionType.Exp, scale=ALPHA
                )
                nc.vector.tensor_tensor(nAT, nAT, nA2, mybir.AluOpType.max)
                nBT = workp.tile([N, GRP * N], F32, tag="nBT")
                nB2 = workp.tile([N, GRP * N], F32, tag="nB2")
                nc.scalar.activation(nBT, accB, mybir.ActivationFunctionType.Exp)
                nc.scalar.activation(
                    nB2, accB, mybir.ActivationFunctionType.Exp, scale=ALPHA
                )
                nc.vector.tensor_tensor(nBT, nBT, nB2, mybir.AluOpType.max)

                # aggregation: outX[i,d] = sum_j nXT[j,i]*h[j,d]; den via ones col
                oA = aggps.tile([N, GRP * D], F32, tag="oA")
                oB = aggps.tile([N, GRP * D], F32, tag="oB")
                den = aggps.tile([N, 2 * GRP], F32, tag="den")
                for b in range(GRP):
                    nsA = nAT[:, b * N:(b + 1) * N]
                    nsB = nBT[:, b * N:(b + 1) * N]
                    hs = hp[:, b * HW:b * HW + D]
                    nc.tensor.matmul(oA[:, b * D:(b + 1) * D], nsA, hs)
                    nc.tensor.matmul(den[:, b:b + 1], nsA, ones_sb)
                    nc.tensor.matmul(oB[:, b * D:(b + 1) * D], nsB, hs)
                    nc.tensor.matmul(den[:, GRP + b:GRP + b + 1], nsB, ones_sb)

                rec = workp.tile([N, 2 * GRP], F32, tag="rec")
                nc.vector.reciprocal(rec, den)
                out4 = workp.tile([N, GRP * D], F32, tag="out4")
                tmp = workp.tile([N, GRP * D], F32, tag="tmp")
                for b in range(GRP):
                    nc.vector.tensor_scalar_mul(
                        tmp[:, b * D:(b + 1) * D],
                        oA[:, b * D:(b + 1) * D],
                        rec[:, b:b + 1],
                    )
                    nc.vector.scalar_tensor_tensor(
                        out4[:, b * D:(b + 1) * D],
                        oB[:, b * D:(b + 1) * D],
                        rec[:, GRP + b:GRP + b + 1],
                        tmp[:, b * D:(b + 1) * D],
                        mybir.AluOpType.mult,
                        mybir.AluOpType.add,
                    )
                nc.sync.dma_start(
                    out=out[:, g * GRP * D:(g + 1) * GRP * D], in_=out4
                )
    nc.compile()
    return nc


def kernel(hidden, adj, beh_adj, A, Bm):
    hidden = np.asarray(hidden, dtype=np.float32)
    adj8 = np.asarray(adj).astype(np.uint8)
    beh8 = np.asarray(beh_adj).astype(np.uint8)
    acat = np.concatenate(
        [np.asarray(A, np.float32), np.asarray(Bm, np.float32)], axis=1
    )
    acat = np.ascontiguousarray(acat)

    if "nc" not in _NC_CACHE:
        _NC_CACHE["nc"] = _build_nc()
    nc = _NC_CACHE["nc"]

    in_maps = []
    for c in range(NCORES):
        sl = slice(c * BPC, (c + 1) * BPC)
        h_c = hidden[sl]                                   # [16,100,128]
        hpT = np.ones((N, BPC, HW), np.float32)
        hpT[:, :, :D] = h_c.transpose(1, 0, 2)
        htr = np.ascontiguousarray(h_c.transpose(2, 0, 1)).reshape(D, BPC * N)
        adt = np.ascontiguousarray(adj8[sl].transpose(2, 0, 1)).reshape(N, BPC * N)
        bet = np.ascontiguousarray(beh8[sl].transpose(2, 0, 1)).reshape(N, BPC * N)
        in_maps.append(
            {
                "hplus": np.ascontiguousarray(hpT).reshape(N, BPC * HW),
                "htr": htr,
                "adjt": adt,
                "beht": bet,
                "acat": acat,
            }
        )

    res = run_bass_kernel_spmd(nc, in_maps, list(range(NCORES)))
    outs = []
    for c in range(NCORES):
        o = res.results[c]["out"].reshape(N, BPC, D).transpose(1, 0, 2)
        outs.append(o)
    return np.ascontiguousarray(np.concatenate(outs, axis=0), dtype=np.float32)


# revision 15
# speedup vs baseline: 1.0018x; 1.0018x over previous
"""LocalAggregator (GAT-style dual-relation message passing) on 8 TRN2 cores.

Math (per batch b, N=100 nodes, D=128):
  e_k[i,j]   = sum_d h[i,d]*h[j,d]*A[d,k]      (k=0..2)   -- symmetric in (i,j)
  b_k[i,j]   = sum_d h[i,d]*h[j,d]*Bm[d,k]     (k=0..8)   -- symmetric
  alpha      = softmax_j( leaky( e_{adj-1} ) masked adj==0 )
  alpha_beh  = softmax_j( leaky( b_{beh-1} ) masked beh==0 )
  out        = 0.5*alpha@h + 0.5*alpha_beh@h

Device strategy (data-parallel, 16 batches/core, groups of 4):
  - planes are symmetric -> select with TRANSPOSED adjacency to directly build
    nT [j, i] (lhsT of the aggregation matmul). No on-chip transposes of planes.
  - no softmax max-subtraction (scores are O(5)); invalid entries get -1e5 via
    acc init so exp()->0.
  - denominator via ones-column matmul; normalization = one fused
    scalar_tensor_tensor per batch.
"""

import numpy as np

import concourse.bass as bass
import concourse.bacc as bacc
import concourse.mybir as mybir
import concourse.tile as tile
from concourse.bass_utils import run_bass_kernel_spmd

F32 = mybir.dt.float32
U8 = mybir.dt.uint8

B, N, D = 128, 100, 128
NCORES = 8
BPC = B // NCORES          # 16 batches per core
GRP = 4                    # batches per group
NGRP = BPC // GRP          # 4 groups
HW = D + 4                 # 132: h row + ones col (+3 pad for alignment)
ALPHA = 0.2
MASKV = -1.0e5

_NC_CACHE = {}


def _build_nc():
    nc = bacc.Bacc()
    hplus = nc.declare_dram_parameter("hplus", [N, BPC * HW], F32, isOutput=False)
    htr = nc.declare_dram_parameter("htr", [D, BPC * N], F32, isOutput=False)
    adjt = nc.declare_dram_parameter("adjt", [N, BPC * N], U8, isOutput=False)
    beht = nc.declare_dram_parameter("beht", [N, BPC * N], U8, isOutput=False)
    acat = nc.declare_dram_parameter("acat", [D, 12], F32, isOutput=False)
    out = nc.declare_dram_parameter("out", [N, BPC * D], F32, isOutput=True)

    with tile.TileContext(nc) as tc:
        with (
            tc.tile_pool(name="const", bufs=1) as constp,
            tc.tile_pool(name="io", bufs=3) as iop,
            tc.tile_pool(name="gk", bufs=4) as gkp,
            tc.tile_pool(name="work", bufs=2) as workp,
            tc.tile_pool(name="eqp", bufs=4) as eqp,
            tc.tile_pool(name="plps", bufs=3, space="PSUM") as plps,
            tc.tile_pool(name="aggps", bufs=1, space="PSUM") as aggps,
        ):
            acat_sb = constp.tile([D, 12], F32)
            nc.sync.dma_start(out=acat_sb, in_=acat[:, :])
            # 2.0 so den = 2*sum and 1/den directly gives the 0.5 blend factor
            ones_sb = constp.tile([N, 1], F32)
            nc.vector.memset(ones_sb, 2.0)

            for g in range(NGRP):
                hp = iop.tile([N, GRP * HW], F32, tag="hp")
                nc.sync.dma_start(out=hp, in_=hplus[:, g * GRP * HW:(g + 1) * GRP * HW])
                adt = iop.tile([N, GRP * N], U8, tag="adt")
                nc.sync.dma_start(out=adt, in_=adjt[:, g * GRP * N:(g + 1) * GRP * N])
                bet = iop.tile([N, GRP * N], U8, tag="bet")
                nc.sync.dma_start(out=bet, in_=beht[:, g * GRP * N:(g + 1) * GRP * N])

                # hT for the 4 batches: [128(d), 4*100(j)], host-pretransposed
                ht4 = iop.tile([D, GRP * N], F32, tag="ht4")
                nc.sync.dma_start(out=ht4, in_=htr[:, g * GRP * N:(g + 1) * GRP * N])

                accA = workp.tile([N, GRP * N], F32, tag="accA")
                nc.vector.memset(accA, MASKV)
                accB = workp.tile([N, GRP * N], F32, tag="accB")
                nc.vector.memset(accB, MASKV)

                for k in range(12):
                    gk = gkp.tile([D, GRP * N], F32, tag="gk")
                    nc.scalar.activation(
                        gk, ht4, mybir.ActivationFunctionType.Copy,
                        scale=acat_sb[:, k:k + 1],
                    )
                    pl = plps.tile([N, GRP * N], F32, tag="pl")
                    for b in range(GRP):
                        nc.tensor.matmul(
                            pl[:, b * N:(b + 1) * N],
                            ht4[:, b * N:(b + 1) * N],
                            gk[:, b * N:(b + 1) * N],
                        )
                    eq = eqp.tile([N, GRP * N], U8, tag="eq")
                    if k < 3:
                        nc.gpsimd.tensor_scalar(
                            eq, adt, k + 1, None, mybir.AluOpType.is_equal
                        )
                        nc.vector.copy_predicated(accA, eq, pl)
                    else:
                        nc.gpsimd.tensor_scalar(
                            eq, bet, k - 2, None, mybir.AluOpType.is_equal
                        )
                        nc.vector.copy_predicated(accB, eq, pl)

                # n = exp(leaky_0.2(acc)) = max(exp(acc), exp(0.2*acc));
                # invalid entries stay exp(-1e5) = 0.  (ACT Lrelu hardcodes
                # slope 0.01, so the max-of-exps identity is used instead.)
                nAT = workp.tile([N, GRP * N], F32, tag="nAT")
                nA2 = workp.tile([N, GRP * N], F32, tag="nA2")
                nc.scalar.activation(nAT, accA, mybir.ActivationFunctionType.Exp)
                nc.scalar.activation(
                    nA2, accA, mybir.ActivationFunctionType.Exp, scale=ALPHA
                )
                nc.vector.tensor_tensor(nAT, nAT, nA2, mybir.AluOpType.max)
                nBT = workp.tile([N, GRP * N], F32, tag="nBT")
                nB2 = workp.tile([N, GRP * N], F32, tag="nB2")
                nc.scalar.activation(nBT, accB, mybir.ActivationFunctionType.Exp)
                nc.scalar.activation(
                    nB2, accB, mybir.ActivationFunctionType.Exp, scale=ALPHA
                )
                nc.vector.tensor_tensor(nBT, nBT, nB2, mybir.AluOpType.max)

                # aggregation: outX[i,d] = sum_j nXT[j,i]*h[j,d]; den via ones col
                oA = aggps.tile([N, GRP * D], F32, tag="oA")
                oB = aggps.tile([N, GRP * D], F32, tag="oB")
                den = aggps.tile([N, 2 * GRP], F32, tag="den")
                for b in range(GRP):
                    nsA = nAT[:, b * N:(b + 1) * N]
                    nsB = nBT[:, b * N:(b + 1) * N]
                    hs = hp[:, b * HW:b * HW + D]
                    nc.tensor.matmul(oA[:, b * D:(b + 1) * D], nsA, hs)
                    nc.tensor.matmul(den[:, b:b + 1], nsA, ones_sb)
                    nc.tensor.matmul(oB[:, b * D:(b + 1) * D], nsB, hs)
                    nc.tensor.matmul(den[:, GRP + b:GRP + b + 1], nsB, ones_sb)

                rec = workp.tile([N, 2 * GRP], F32, tag="rec")
                nc.vector.reciprocal(rec, den)
                out4 = workp.tile([N, GRP * D], F32, tag="out4")
                tmp = workp.tile([N, GRP * D], F32, tag="tmp")
                for b in range(GRP):
                    nc.vector.tensor_scalar_mul(
                        tmp[:, b * D:(b + 1) * D],
                        oA[:, b * D:(b + 1) * D],
                        rec[:, b:b + 1],
                    )
                    nc.vector.scalar_tensor_tensor(
                        out4[:, b * D:(b + 1) * D],
                        oB[:, b * D:(b + 1) * D],
                        rec[:, GRP + b:GRP + b + 1],
                        tmp[:, b * D:(b + 1) * D],
                        mybir.AluOpType.mult,
                        mybir.AluOpType.add,
                    )
                nc.sync.dma_start(
                    out=out[:, g * GRP * D:(g + 1) * GRP * D], in_=out4
                )
    nc.compile()
    return nc


def kernel(hidden, adj, beh_adj, A, Bm):
    hidden = np.asarray(hidden, dtype=np.float32)
    adj8 = np.asarray(adj).astype(np.uint8)
    beh8 = np.asarray(beh_adj).astype(np.uint8)
    acat = np.concatenate(
        [np.asarray(A, np.float32), np.asarray(Bm, np.float32)], axis=1
    )
    acat = np.ascontiguousarray(acat)

    if "nc" not in _NC_CACHE:
        _NC_CACHE["nc"] = _build_nc()
    nc = _NC_CACHE["nc"]

    in_maps = []
    for c in range(NCORES):
        sl = slice(c * BPC, (c + 1) * BPC)
        h_c = hidden[sl]                                   # [16,100,128]
        hpT = np.ones((N, BPC, HW), np.float32)
        hpT[:, :, :D] = h_c.transpose(1, 0, 2)
        htr = np.ascontiguousarray(h_c.transpose(2, 0, 1)).reshape(D, BPC * N)
        adt = np.ascontiguousarray(adj8[sl].transpose(2, 0, 1)).reshape(N, BPC * N)
        bet = np.ascontiguousarray(beh8[sl].transpose(2, 0, 1)).reshape(N, BPC * N)
        in_maps.append(
            {
                "hplus": np.ascontiguousarray(hpT).reshape(N, BPC * HW),
                "htr": htr,
                "adjt": adt,
                "beht": bet,
                "acat": acat,
            }
        )

    res = run_bass_kernel_spmd(nc, in_maps, list(range(NCORES)))
    outs = []
    for c in range(NCORES):
        o = res.results[c]["out"].reshape(N, BPC, D).transpose(1, 0, 2)
        outs.append(o)
    return np.ascontiguousarray(np.concatenate(outs, axis=0), dtype=np.float32)
